# revision 16
# baseline (speedup 1.0000x reference)
"""CapsuleLayer kernel for 8 Trainium2 NeuronCores.

Math: with b0 = 0, softmax(b0, axis=1) is exactly uniform (1/N), so
outputs[b,i,k] = squash_k((1/N) * sum_j inputs_hat[b,j,k]) independent of i.
The b-update keeps b constant along axis 1, so softmax stays exactly uniform
and all routing iterations return the same outputs. Hence:

    Wsum[m,k] = sum_j W[j,m,k]
    v[b,k]    = (1/N) * (inputs @ Wsum)[b,k]
    out[b,i,k] = squash_k(v)[b,k]          (broadcast over i)

Kernel 1 (m-sharded): core c reduces W[:, 32c:32c+32, :] over j -> Wsum rows.
Loads are split across both HWDGE rings (sync=j-low, scalar=j-high) so the
per-DMA completion latency of one ring hides under the other's data. DVE adds
the halves, ones-matmuls fold the 128 partitions, and each 512-col group
lands on its OWN PSUM partition (16 groups -> [16, 512] PSUM tile) so the
PSUM->SBUF copies are multi-partition (a [1, 8192] row costs ~9 us of
single-lane DVE; [16, 512] costs ~1 us total).

Kernel 2 (batch-sharded): core c computes squash((inputs_c @ Wsum)/N) and
broadcast-writes its [64, 256, 256] output slice. The squashed row is
materialized 8x in SBUF so output DMA descriptors are 8 KB (a pure stride-0
256-elem source yields 1 KB descriptors, which caps SDMA engines at ~21 GB/s
and starves them on HWDGE descriptor generation).
"""

import numpy as np

import concourse.bass as bass
import concourse.mybir as mybir
import concourse.tile as tile
from concourse.ap import AP
from concourse.bass_utils import run_bass_kernel_spmd

F32 = mybir.dt.float32

B, N = 512, 256
NCORES = 8
BPC = B // NCORES  # 64 batch rows per core (kernel 2)
MPC = N // NCORES  # 32 m rows per core (kernel 1)
EPS = 1e-7
SREP = 8           # squash-row copies materialized in SBUF (kernel 2)

_CACHE = {}


def _fix_multiwait(nc, maxw=1):
    """This walrus build rejects instructions carrying more than one sync
    wait ("Too many sync wait commands"). Hoist extra waits into standalone
    single-wait EventSemaphore instructions on the same engine, placed
    immediately before the offender."""
    ctr = 0
    for fn in nc.m.functions:
        for bb in fn.blocks:
            out = []
            for ins in bb.instructions:
                si = ins.sync_info
                if si is not None and len(si.on_wait) > maxw:
                    waits = list(si.on_wait)
                    for w in waits[:-maxw]:
                        ctr += 1
                        ev = mybir.InstEventSemaphore(
                            name=f"mwsplit-{ctr}",
                            engine=ins.engine,
                            ins=[],
                            outs=[],
                            sync_info=mybir.SyncInfo(on_wait=[w], on_update=[]),
                        )
                        nc.register_instruction(ev, overwrite=True)
                        out.append(ev)
                    si.on_wait = waits[-maxw:]
                    ins.sync_info = si
                out.append(ins)
            bb.instructions[:] = out
    return nc

# Exec times (ns) of the last traced run, for test harnesses.
LAST_EXEC_NS = {"k1": None, "k2": None}


def _build_k1():
    """Reduce the per-core W slice over j.

    Input  w_in [256 (j), 8192 (m_local*256 + k)]  (= W[:, mslice, :] flat)
    Output wsum_part [1, 8192]  (= Wsum[mslice, :] flat)

    Per chunk: DMA the two j-halves on separate HWDGE rings, DVE-add them
    (j 256->128), ones-matmul each 512-col group onto its own PSUM partition,
    copy the chunk's PSUM partitions out at once, and store them via SWDGE so
    the load rings are never interrupted.
    """
    nc = bass.Bass()
    FREE = MPC * N    # 8192
    MMF = 512         # moving free dim per matmul (one PSUM bank of f32)

    w = nc.dram_tensor("w_in", [N, FREE], F32, kind="ExternalInput")
    wsum = nc.dram_tensor("wsum_part", [1, FREE], F32, kind="ExternalOutput")

    # Strictly decreasing chunk sizes: chunk c's (add -> matmul -> copy ->
    # store) train runs while chunk c+1 is still loading, and the LAST chunk
    # is small so the post-load drain is short.
    CHUNKS = [2048, 2048, 1536, 1024, 768, 512, 256]
    assert sum(CHUNKS) == FREE

    with tile.TileContext(nc) as tc:
        with (
            tc.tile_pool(name="singles", bufs=1) as singles,
            tc.tile_pool(name="psum", bufs=8, space="PSUM") as psum_pool,
        ):
            ones = singles.tile([128, 1], F32)
            nc.vector.memset(ones[:], 1.0)
            acc = singles.tile([1, FREE], F32)
            # Touch ACT once so its function-table load (~1.3 us) happens
            # during the load phase, not inside the first copy.
            warm = singles.tile([1, 1], F32)
            nc.scalar.activation(
                out=warm[:], in_=ones[0:1, 0:1],
                func=mybir.ActivationFunctionType.Copy,
            )

            # Issue every load up front so each HWDGE ring streams its 4 MiB
            # back-to-back regardless of compute progress.
            tas, tbs = [], []
            off = 0
            for ci, chunk in enumerate(CHUNKS):
                sl = slice(off, off + chunk)
                ta = singles.tile([128, chunk], F32, tag=f"ta{ci}")
                nc.sync.dma_start(out=ta[:], in_=w[0:128, sl])
                tas.append(ta)
                tb = singles.tile([128, chunk], F32, tag=f"tb{ci}")
                nc.scalar.dma_start(out=tb[:], in_=w[128:256, sl])
                tbs.append(tb)
                off += chunk

            off = 0
            gg = 0  # global group counter, for DVE/ACT copy alternation
            last = len(CHUNKS) - 1
            for ci, chunk in enumerate(CHUNKS):
                sl = slice(off, off + chunk)
                ts = singles.tile([128, chunk], F32, tag=f"ts{ci}")
                nc.vector.tensor_add(ts[:], tas[ci][:], tbs[ci][:])
                g_off = 0
                while g_off < chunk:
                    mmf = min(MMF, chunk - g_off)
                    ps = psum_pool.tile([1, mmf], F32)
                    nc.tensor.matmul(
                        ps[:], lhsT=ones[:], rhs=ts[:, g_off:g_off + mmf],
                        start=True, stop=True,
                    )
                    osl = slice(off + g_off, off + g_off + mmf)
                    g_off += mmf
                    # All PSUM->SBUF copies go to ACT: DVE's add chain is the
                    # tail-critical resource, ACT is otherwise idle.
                    nc.scalar.activation(
                        out=acc[0:1, osl], in_=ps[:],
                        func=mybir.ActivationFunctionType.Copy,
                    )
                    gg += 1
                # Stores ride the HWDGE rings: issued (program-order) after
                # all loads, so ring FIFO never delays a load, and HWDGE
                # avoids SWDGE's ~1-4 us Q7 descriptor latency.
                eng = nc.sync if ci % 2 == 0 else nc.scalar
                eng.dma_start(out=wsum[0:1, sl], in_=acc[0:1, sl])
                off += chunk
    return nc


def _build_k2():
    """Per-core: u = inputs_c @ Wsum, s = squash(u/N), broadcast-write output.

    Inputs  xt   [256 (m), 64 (b)]   (= inputs_c.T)
            wsum [256 (m), 256 (k)]
    Output  out  [BPC*N*N] flat = out[b, i, k] with value s[b, k].

    PSUM partition q = 2*b + ihalf (interleaved duplicate of b), so partition
    q owns the contiguous 128 KB output block out[q*32768 : (q+1)*32768]
    (= 128 i-rows of 256). The squashed row is replicated SREP times in SBUF
    (DVE + ACT split) so each output descriptor moves SREP KB.
    """
    nc = bass.Bass()
    xd = nc.dram_tensor("xd", [N, 2 * BPC], F32, kind="ExternalInput")
    ws = nc.dram_tensor("wsum", [N, N], F32, kind="ExternalInput")
    out = nc.dram_tensor("out", [BPC * N * N], F32, kind="ExternalOutput")

    WIDE = SREP * N             # 2048 elements = 8 materialized i-rows
    NREP_OUT = 128 // SREP      # stride-0 outer reps per partition, total
    HREP = NREP_OUT // 2        # outer reps per DMA (one per HWDGE ring)

    with tile.TileContext(nc) as tc:
        with (
            tc.tile_pool(name="sb", bufs=1) as sb,
            tc.tile_pool(name="psum", bufs=1, space="PSUM") as psum_pool,
        ):
            # Load the (host-pre-duplicated) inputs_c.T and Wsum halves
            # (contraction dim m on partitions) on the sync HWDGE ring,
            # ordered so the first matmul's operands land first.
            ws0 = sb.tile([128, N], F32)
            nc.sync.dma_start(out=ws0[:], in_=ws[0:128, :])
            xd0 = sb.tile([128, 2 * BPC], F32)
            nc.sync.dma_start(out=xd0[:], in_=xd[0:128, :])
            ws1 = sb.tile([128, N], F32)
            nc.sync.dma_start(out=ws1[:], in_=ws[128:256, :])
            xd1 = sb.tile([128, 2 * BPC], F32)
            nc.sync.dma_start(out=xd1[:], in_=xd[128:256, :])

            # u[q, k] = sum_m inputs_c[q//2, m] * Wsum[m, k]
            u = psum_pool.tile([128, N], F32)
            nc.tensor.matmul(u[:], lhsT=xd0[:], rhs=ws0[:], start=True, stop=False)
            nc.tensor.matmul(u[:], lhsT=xd1[:], rhs=ws1[:], start=False, stop=True)

            # squash: v = u/N; s2 = sum_k v^2; s = v * s2/(1+s2)/sqrt(s2+eps)
            #       = u * factor,  factor = s2/(1+s2)/sqrt(s2+eps)/N
            sq = sb.tile([128, N], F32)
            s2 = sb.tile([128, 1], F32)
            nc.scalar.activation(
                out=sq[:], in_=u[:], func=mybir.ActivationFunctionType.Square,
                scale=1.0 / N, accum_out=s2[:],
            )
            eps_t = sb.tile([128, 1], F32)
            nc.vector.memset(eps_t[:], EPS)
            r = sb.tile([128, 1], F32)
            nc.scalar.activation(
                out=r[:], in_=s2[:], func=mybir.ActivationFunctionType.Sqrt,
                bias=eps_t[:],
            )
            den = sb.tile([128, 1], F32)
            nc.vector.scalar_tensor_tensor(
                den[:], s2[:], 1.0, r[:],
                op0=mybir.AluOpType.add, op1=mybir.AluOpType.mult,
            )
            rec = sb.tile([128, 1], F32)
            nc.vector.reciprocal(rec[:], den[:])
            fac = sb.tile([128, 1], F32)
            nc.vector.scalar_tensor_tensor(
                fac[:], s2[:], 1.0 / N, rec[:],
                op0=mybir.AluOpType.mult, op1=mybir.AluOpType.mult,
            )

            # Materialize s[q, k] = u[q, k] * fac[q] replicated SREP/2 times
            # into TWO tiles, one per output DMA: the DVE tile gates only the
            # sync-ring DMA and the ACT tile only the scalar-ring DMA, so
            # each write starts as soon as its own half is ready. Both source
            # u directly (stride-0 reps) — all rows are identical anyway.
            HW_ = WIDE // 2  # 1024 elements = 4 materialized i-rows

            def u_rep(nrep):
                return AP(
                    tensor=u.tensor,
                    offset=u[:].offset,
                    ap=[u[:].ap[0], [0, nrep], [1, N]],
                )

            s_a = sb.tile([128, HW_], F32)
            nc.vector.tensor_scalar(
                s_a[:], u_rep(SREP // 2), fac[:], None, mybir.AluOpType.mult
            )
            s_b = sb.tile([128, HW_], F32)
            nc.scalar.activation(
                out=s_b[:], in_=u_rep(SREP // 2),
                func=mybir.ActivationFunctionType.Copy, scale=fac[:],
            )

            # Output writes: partition q writes out[q*32768 + o*1024 + t] =
            # s_x[q, t] (16 outer reps per ring). SDMA engine 15 (serving
            # partitions 92-95/124-127) intermittently runs ~18% slow and
            # then gates the whole burst; those partitions only write 13/16
            # reps directly, and the last 3 reps are sourced from aux copies
            # of their rows placed on partitions {0,4,8,12}/{64,68,72,76}
            # (engines 0,2,4,6 and 1,3,5,7). Aux copies are built by SWDGE
            # (idle) while the main burst runs.
            KEEP = 13
            REPS_R = 2 * HREP  # 16 outer reps per ring per partition
            PP = HW_           # partition pitch of s_x / aux, elements

            def srcap(s_x, p0, np_, nrep, pstride=1):
                return AP(
                    tensor=s_x.tensor,
                    offset=s_x[:].offset + p0 * PP,
                    ap=[[pstride * PP, np_], [0, nrep], [1, HW_]],
                )

            def dstap(g, p0, np_, r0, nrep):
                return AP(
                    tensor=out,
                    offset=p0 * 128 * N + g * HREP * WIDE + r0 * HW_,
                    ap=[[128 * N, np_], [HW_, nrep], [1, HW_]],
                )

            for g, (eng, s_x) in enumerate(((nc.sync, s_a), (nc.scalar, s_b))):
                aux = sb.tile([128, HW_], F32, tag=f"aux{g}")
                for (slowb, auxb) in ((92, 0), (124, 64)):
                    bsrc = s_x[slowb:slowb + 4, :]
                    bdst = AP(
                        tensor=aux.tensor,
                        offset=aux[:].offset + auxb * PP,
                        ap=[[4 * PP, 4], [1, HW_]],
                    )
                    nc.gpsimd.dma_start(out=bdst, in_=bsrc)
                eng.dma_start(out=dstap(g, 0, 92, 0, REPS_R),
                              in_=srcap(s_x, 0, 92, REPS_R))
                eng.dma_start(out=dstap(g, 92, 4, 0, KEEP),
                              in_=srcap(s_x, 92, 4, KEEP))
                eng.dma_start(out=dstap(g, 96, 28, 0, REPS_R),
                              in_=srcap(s_x, 96, 28, REPS_R))
                eng.dma_start(out=dstap(g, 124, 4, 0, KEEP),
                              in_=srcap(s_x, 124, 4, KEEP))
                eng.dma_start(out=dstap(g, 92, 4, KEEP, REPS_R - KEEP),
                              in_=srcap(aux, 0, 4, REPS_R - KEEP, pstride=4))
                eng.dma_start(out=dstap(g, 124, 4, KEEP, REPS_R - KEEP),
                              in_=srcap(aux, 64, 4, REPS_R - KEEP, pstride=4))
    return nc


def _run(nc, in_maps, core_ids, trace):
    if trace:
        try:
            return run_bass_kernel_spmd(nc, in_maps, core_ids, trace=True)
        except Exception as e:  # noqa: BLE001
            print(f"kernel: trace run failed ({e}); rerunning without trace")
    return run_bass_kernel_spmd(nc, in_maps, core_ids, trace=False)


def _get(name):
    if name not in _CACHE:
        _CACHE[name] = _fix_multiwait(_build_k1() if name == "k1" else _build_k2())
    return _CACHE[name]


def kernel(inputs: np.ndarray, W: np.ndarray, trace: bool = False) -> np.ndarray:
    inputs = np.ascontiguousarray(inputs, dtype=np.float32)
    W = np.ascontiguousarray(W, dtype=np.float32)
    core_ids = list(range(NCORES))

    # ---- kernel 1: Wsum rows, m-sharded ----
    k1 = _get("k1")
    in_maps1 = [
        {
            "w_in": np.ascontiguousarray(
                W[:, c * MPC:(c + 1) * MPC, :]
            ).reshape(N, MPC * N)
        }
        for c in core_ids
    ]
    res1 = _run(k1, in_maps1, core_ids, trace)
    LAST_EXEC_NS["k1"] = res1.exec_time_ns
    wsum = np.concatenate(
        [res1.results[c]["wsum_part"].reshape(MPC, N) for c in core_ids], axis=0
    )  # [256, 256]

    # ---- kernel 2: squash + broadcast write, batch-sharded ----
    k2 = _get("k2")
    xt_full = inputs.T  # [256, 512]
    in_maps2 = [
        {
            # xd[:, 2b + d] = inputs_c.T[:, b] — PSUM partition q = 2b + d.
            "xd": np.ascontiguousarray(
                np.repeat(xt_full[:, c * BPC:(c + 1) * BPC], 2, axis=1)
            ),
            "wsum": wsum,
        }
        for c in core_ids
    ]
    res2 = _run(k2, in_maps2, core_ids, trace)
    LAST_EXEC_NS["k2"] = res2.exec_time_ns
    out = np.concatenate(
        [res2.results[c]["out"].reshape(BPC, N, N) for c in core_ids], axis=0
    )
    return out


# revision 17
# speedup vs baseline: 2.0482x; 2.0482x over previous
"""CapsuleLayer kernel for 8 Trainium2 NeuronCores.

Math: with b0 = 0, softmax(b0, axis=1) is exactly uniform (1/N), so
outputs[b,i,k] = squash_k((1/N) * sum_j inputs_hat[b,j,k]) independent of i.
The b-update keeps b constant along axis 1, so softmax stays exactly uniform
and all routing iterations return the same outputs. Hence:

    Wsum[m,k] = sum_j W[j,m,k]
    v[b,k]    = (1/N) * (inputs @ Wsum)[b,k]
    out[b,i,k] = squash_k(v)[b,k]          (broadcast over i)

Kernel 1 (m-sharded): core c reduces W[:, 32c:32c+32, :] over j -> Wsum rows.
Loads are split across both HWDGE rings (sync=j-low, scalar=j-high) so the
per-DMA completion latency of one ring hides under the other's data. DVE adds
the halves, ones-matmuls fold the 128 partitions, and each 512-col group
lands on its OWN PSUM partition (16 groups -> [16, 512] PSUM tile) so the
PSUM->SBUF copies are multi-partition (a [1, 8192] row costs ~9 us of
single-lane DVE; [16, 512] costs ~1 us total).

Kernel 2 (batch-sharded): core c computes squash((inputs_c @ Wsum)/N) and
broadcast-writes its [64, 256, 256] output slice. The squashed row is
materialized 8x in SBUF so output DMA descriptors are 8 KB (a pure stride-0
256-elem source yields 1 KB descriptors, which caps SDMA engines at ~21 GB/s
and starves them on HWDGE descriptor generation).
"""

import numpy as np

import concourse.bass as bass
import concourse.mybir as mybir
import concourse.tile as tile
from concourse.ap import AP
from concourse.bass_utils import run_bass_kernel_spmd

F32 = mybir.dt.float32

B, N = 512, 256
NCORES = 8
BPC = B // NCORES  # 64 batch rows per core (kernel 2)
MPC = N // NCORES  # 32 m rows per core (kernel 1)
EPS = 1e-7
SREP = 8           # squash-row copies materialized in SBUF (kernel 2)

_CACHE = {}


def _fix_multiwait(nc, maxw=1):
    """This walrus build rejects instructions carrying more than one sync
    wait ("Too many sync wait commands"). Hoist extra waits into standalone
    single-wait EventSemaphore instructions on the same engine, placed
    immediately before the offender."""
    ctr = 0
    for fn in nc.m.functions:
        for bb in fn.blocks:
            out = []
            for ins in bb.instructions:
                si = ins.sync_info
                if si is not None and len(si.on_wait) > maxw:
                    waits = list(si.on_wait)
                    for w in waits[:-maxw]:
                        ctr += 1
                        ev = mybir.InstEventSemaphore(
                            name=f"mwsplit-{ctr}",
                            engine=ins.engine,
                            ins=[],
                            outs=[],
                            sync_info=mybir.SyncInfo(on_wait=[w], on_update=[]),
                        )
                        nc.register_instruction(ev, overwrite=True)
                        out.append(ev)
                    si.on_wait = waits[-maxw:]
                    ins.sync_info = si
                out.append(ins)
            bb.instructions[:] = out
    return nc

# Exec times (ns) of the last traced run, for test harnesses.
LAST_EXEC_NS = {"k1": None, "k2": None}


def _build_k1():
    """Reduce the per-core W slice over j.

    Input  w_in [256 (j), 8192 (m_local*256 + k)]  (= W[:, mslice, :] flat)
    Output wsum_part [1, 8192]  (= Wsum[mslice, :] flat)

    Per chunk: DMA the two j-halves on separate HWDGE rings, DVE-add them
    (j 256->128), ones-matmul each 512-col group onto its own PSUM partition,
    copy the chunk's PSUM partitions out at once, and store them via SWDGE so
    the load rings are never interrupted.
    """
    nc = bass.Bass()
    FREE = MPC * N    # 8192
    MMF = 512         # moving free dim per matmul (one PSUM bank of f32)

    w = nc.dram_tensor("w_in", [N, FREE], F32, kind="ExternalInput")
    wsum = nc.dram_tensor("wsum_part", [1, FREE], F32, kind="ExternalOutput")

    # Strictly decreasing chunk sizes: chunk c's (add -> matmul -> copy ->
    # store) train runs while chunk c+1 is still loading, and the LAST chunk
    # is small so the post-load drain is short.
    CHUNKS = [2048, 2048, 1536, 1024, 768, 512, 256]
    assert sum(CHUNKS) == FREE

    with tile.TileContext(nc) as tc:
        with (
            tc.tile_pool(name="singles", bufs=1) as singles,
            tc.tile_pool(name="psum", bufs=8, space="PSUM") as psum_pool,
        ):
            ones = singles.tile([128, 1], F32)
            nc.vector.memset(ones[:], 1.0)
            acc = singles.tile([1, FREE], F32)
            # Touch ACT once so its function-table load (~1.3 us) happens
            # during the load phase, not inside the first copy.
            warm = singles.tile([1, 1], F32)
            nc.scalar.activation(
                out=warm[:], in_=ones[0:1, 0:1],
                func=mybir.ActivationFunctionType.Copy,
            )

            # Issue every load up front so each HWDGE ring streams its 4 MiB
            # back-to-back regardless of compute progress.
            tas, tbs = [], []
            off = 0
            for ci, chunk in enumerate(CHUNKS):
                sl = slice(off, off + chunk)
                ta = singles.tile([128, chunk], F32, tag=f"ta{ci}")
                nc.sync.dma_start(out=ta[:], in_=w[0:128, sl])
                tas.append(ta)
                tb = singles.tile([128, chunk], F32, tag=f"tb{ci}")
                nc.scalar.dma_start(out=tb[:], in_=w[128:256, sl])
                tbs.append(tb)
                off += chunk

            off = 0
            gg = 0  # global group counter, for DVE/ACT copy alternation
            last = len(CHUNKS) - 1
            for ci, chunk in enumerate(CHUNKS):
                sl = slice(off, off + chunk)
                ts = singles.tile([128, chunk], F32, tag=f"ts{ci}")
                nc.vector.tensor_add(ts[:], tas[ci][:], tbs[ci][:])
                g_off = 0
                while g_off < chunk:
                    mmf = min(MMF, chunk - g_off)
                    ps = psum_pool.tile([1, mmf], F32)
                    nc.tensor.matmul(
                        ps[:], lhsT=ones[:], rhs=ts[:, g_off:g_off + mmf],
                        start=True, stop=True,
                    )
                    osl = slice(off + g_off, off + g_off + mmf)
                    g_off += mmf
                    # All PSUM->SBUF copies go to ACT: DVE's add chain is the
                    # tail-critical resource, ACT is otherwise idle.
                    nc.scalar.activation(
                        out=acc[0:1, osl], in_=ps[:],
                        func=mybir.ActivationFunctionType.Copy,
                    )
                    gg += 1
                # Stores ride the HWDGE rings: issued (program-order) after
                # all loads, so ring FIFO never delays a load, and HWDGE
                # avoids SWDGE's ~1-4 us Q7 descriptor latency.
                eng = nc.sync if ci % 2 == 0 else nc.scalar
                eng.dma_start(out=wsum[0:1, sl], in_=acc[0:1, sl])
                off += chunk
    return nc


def _build_k2():
    """Per-core: u = inputs_c @ Wsum, s = squash(u/N), broadcast-write output.

    Inputs  xt   [256 (m), 64 (b)]   (= inputs_c.T)
            wsum [256 (m), 256 (k)]
    Output  out  [BPC*N*N] flat = out[b, i, k] with value s[b, k].

    PSUM partition q = 2*b + ihalf (interleaved duplicate of b), so partition
    q owns the contiguous 128 KB output block out[q*32768 : (q+1)*32768]
    (= 128 i-rows of 256). The squashed row is replicated SREP times in SBUF
    (DVE + ACT split) so each output descriptor moves SREP KB.
    """
    nc = bass.Bass()
    xd = nc.dram_tensor("xd", [N, 2 * BPC], F32, kind="ExternalInput")
    ws = nc.dram_tensor("wsum", [N, N], F32, kind="ExternalInput")
    out = nc.dram_tensor("out", [BPC * N * N], F32, kind="ExternalOutput")

    WIDE = SREP * N             # 2048 elements = 8 materialized i-rows
    NREP_OUT = 128 // SREP      # stride-0 outer reps per partition, total
    HREP = NREP_OUT // 2        # outer reps per DMA (one per HWDGE ring)

    with tile.TileContext(nc) as tc:
        with (
            tc.tile_pool(name="sb", bufs=1) as sb,
            tc.tile_pool(name="psum", bufs=1, space="PSUM") as psum_pool,
        ):
            # Load the (host-pre-duplicated) inputs_c.T and Wsum halves
            # (contraction dim m on partitions) on the sync HWDGE ring,
            # ordered so the first matmul's operands land first.
            ws0 = sb.tile([128, N], F32)
            nc.sync.dma_start(out=ws0[:], in_=ws[0:128, :])
            xd0 = sb.tile([128, 2 * BPC], F32)
            nc.sync.dma_start(out=xd0[:], in_=xd[0:128, :])
            ws1 = sb.tile([128, N], F32)
            nc.sync.dma_start(out=ws1[:], in_=ws[128:256, :])
            xd1 = sb.tile([128, 2 * BPC], F32)
            nc.sync.dma_start(out=xd1[:], in_=xd[128:256, :])

            # u[q, k] = sum_m inputs_c[q//2, m] * Wsum[m, k]
            u = psum_pool.tile([128, N], F32)
            nc.tensor.matmul(u[:], lhsT=xd0[:], rhs=ws0[:], start=True, stop=False)
            nc.tensor.matmul(u[:], lhsT=xd1[:], rhs=ws1[:], start=False, stop=True)

            # squash: v = u/N; s2 = sum_k v^2; s = v * s2/(1+s2)/sqrt(s2+eps)
            #       = u * factor,  factor = s2/(1+s2)/sqrt(s2+eps)/N
            sq = sb.tile([128, N], F32)
            s2 = sb.tile([128, 1], F32)
            nc.scalar.activation(
                out=sq[:], in_=u[:], func=mybir.ActivationFunctionType.Square,
                scale=1.0 / N, accum_out=s2[:],
            )
            eps_t = sb.tile([128, 1], F32)
            nc.vector.memset(eps_t[:], EPS)
            r = sb.tile([128, 1], F32)
            nc.scalar.activation(
                out=r[:], in_=s2[:], func=mybir.ActivationFunctionType.Sqrt,
                bias=eps_t[:],
            )
            den = sb.tile([128, 1], F32)
            nc.vector.scalar_tensor_tensor(
                den[:], s2[:], 1.0, r[:],
                op0=mybir.AluOpType.add, op1=mybir.AluOpType.mult,
            )
            rec = sb.tile([128, 1], F32)
            nc.vector.reciprocal(rec[:], den[:])
            fac = sb.tile([128, 1], F32)
            nc.vector.scalar_tensor_tensor(
                fac[:], s2[:], 1.0 / N, rec[:],
                op0=mybir.AluOpType.mult, op1=mybir.AluOpType.mult,
            )

            # Materialize s[q, k] = u[q, k] * fac[q] replicated SREP/2 times
            # into TWO tiles, one per output DMA: the DVE tile gates only the
            # sync-ring DMA and the ACT tile only the scalar-ring DMA, so
            # each write starts as soon as its own half is ready. Both source
            # u directly (stride-0 reps) — all rows are identical anyway.
            HW_ = WIDE // 2  # 1024 elements = 4 materialized i-rows

            def u_rep(nrep):
                return AP(
                    tensor=u.tensor,
                    offset=u[:].offset,
                    ap=[u[:].ap[0], [0, nrep], [1, N]],
                )

            s_a = sb.tile([128, HW_], F32)
            nc.vector.tensor_scalar(
                s_a[:], u_rep(SREP // 2), fac[:], None, mybir.AluOpType.mult
            )
            s_b = sb.tile([128, HW_], F32)
            nc.scalar.activation(
                out=s_b[:], in_=u_rep(SREP // 2),
                func=mybir.ActivationFunctionType.Copy, scale=fac[:],
            )

            # Two 8 MiB output DMAs (one per HWDGE ring). Partition q writes
            # out[q*32768 + o*1024 + t] = s_x[q, t]; descriptors are
            # SREP/2 KB each.
            for g, (eng, s_x) in enumerate(((nc.sync, s_a), (nc.scalar, s_b))):
                src = AP(
                    tensor=s_x.tensor,
                    offset=s_x[:].offset,
                    ap=[s_x[:].ap[0], [0, 2 * HREP], [1, HW_]],
                )
                dst = AP(
                    tensor=out,
                    offset=g * HREP * WIDE,
                    ap=[[128 * N, 128], [HW_, 2 * HREP], [1, HW_]],
                )
                eng.dma_start(out=dst, in_=src)
    return nc


def _run(nc, in_maps, core_ids, trace):
    if trace:
        try:
            return run_bass_kernel_spmd(nc, in_maps, core_ids, trace=True)
        except Exception as e:  # noqa: BLE001
            print(f"kernel: trace run failed ({e}); rerunning without trace")
    return run_bass_kernel_spmd(nc, in_maps, core_ids, trace=False)


def _get(name):
    if name not in _CACHE:
        _CACHE[name] = _fix_multiwait(_build_k1() if name == "k1" else _build_k2())
    return _CACHE[name]


def kernel(inputs: np.ndarray, W: np.ndarray, trace: bool = False) -> np.ndarray:
    inputs = np.ascontiguousarray(inputs, dtype=np.float32)
    W = np.ascontiguousarray(W, dtype=np.float32)
    core_ids = list(range(NCORES))

    # ---- kernel 1: Wsum rows, m-sharded ----
    k1 = _get("k1")
    in_maps1 = [
        {
            "w_in": np.ascontiguousarray(
                W[:, c * MPC:(c + 1) * MPC, :]
            ).reshape(N, MPC * N)
        }
        for c in core_ids
    ]
    res1 = _run(k1, in_maps1, core_ids, trace)
    LAST_EXEC_NS["k1"] = res1.exec_time_ns
    wsum = np.concatenate(
        [res1.results[c]["wsum_part"].reshape(MPC, N) for c in core_ids], axis=0
    )  # [256, 256]

    # ---- kernel 2: squash + broadcast write, batch-sharded ----
    k2 = _get("k2")
    xt_full = inputs.T  # [256, 512]
    in_maps2 = [
        {
            # xd[:, 2b + d] = inputs_c.T[:, b] — PSUM partition q = 2b + d.
            "xd": np.ascontiguousarray(
                np.repeat(xt_full[:, c * BPC:(c + 1) * BPC], 2, axis=1)
            ),
            "wsum": wsum,
        }
        for c in core_ids
    ]
    res2 = _run(k2, in_maps2, core_ids, trace)
    LAST_EXEC_NS["k2"] = res2.exec_time_ns
    out = np.concatenate(
        [res2.results[c]["out"].reshape(BPC, N, N) for c in core_ids], axis=0
    )
    return out
